# revision 12
# baseline (speedup 1.0000x reference)
"""Trainium2 Bass kernel for a pre-norm transformer block (attention + MLP).

Sharding: pure data-parallel over 8 cores. Core c handles batch b=c//2 and
query-row half rh=c%2 (512 tokens). K/V are computed for the full 1024-token
batch on every core (duplicated across the pair) so no collectives are needed.

Device layout: activations are feature-major (features on partitions, tokens
on the free dim). All attention-side GEMMs (Q/K/V/O projections and attn@V)
run in fp8-e4m3 with DoubleRow perf mode (256-deep contraction per matmul);
the MLP stays bf16 for accuracy. LayerNorm stats use float32r ones-matmul
partition reductions; softmax uses exp(sim)*mask01 (exact: masked logits
contribute exp=0) with per-query sums from a ones-column appended to V.
V/O biases are folded into the residual on the host (bo' = bo + bv @ wo).
"""

import numpy as np
import ml_dtypes
from contextlib import ExitStack

import concourse.bass as bass
from concourse.bacc import Bacc
import concourse.tile as tile
from concourse import mybir
from concourse.bass_utils import run_bass_kernel_spmd

F32 = mybir.dt.float32
F32R = mybir.dt.float32r
BF16 = mybir.dt.bfloat16
FP8 = mybir.dt.float8e4
AF = mybir.ActivationFunctionType
ALU = mybir.AluOpType
DR = mybir.MatmulPerfMode.DoubleRow
BFNP = ml_dtypes.bfloat16
F8NP = ml_dtypes.float8_e4m3

B, N, C = 4, 1024, 1024
H, D = 16, 64
DFF = 4096
R = 512          # own query rows per core
P = 128
KC = C // P      # 8 feature k-tiles
K2 = KC // 2     # 4 DoubleRow pair tiles
NT = N // P      # 8 token tiles
EPS = 1e-6

# fp8 scale factors (powers of two; validated in numpy simulation)
S_XN = 16.0      # xn (LN output)
S_WQ = 2048.0    # wq * d^-0.5
S_W = 256.0      # wk, wv, wo
S_Q = 256.0      # qt
S_K = 8.0        # kt
S_V = 16.0       # vt
S_O = 64.0       # ot (normalized attn out)

_CACHE: dict = {}


def _build():
    nc = Bacc()
    io = {}
    io["xT"] = nc.dram_tensor("xT", [C, N], F32R, kind="ExternalInput")
    io["xTb"] = nc.dram_tensor("xTb", [C, R], F32, kind="ExternalInput")
    io["mskT"] = nc.dram_tensor("mskT", [N, R], FP8, kind="ExternalInput")
    for nm in ["wq8", "wk8", "wv8", "wo8"]:
        io[nm] = nc.dram_tensor(nm, [C // 2, 2 * C], FP8, kind="ExternalInput")
    io["w1"] = nc.dram_tensor("w1", [C, DFF], BF16, kind="ExternalInput")
    io["w2"] = nc.dram_tensor("w2", [DFF, C], BF16, kind="ExternalInput")
    for nm, n_ in [("bq", C), ("bk", C), ("b1", DFF), ("b2", C)]:
        io[nm] = nc.dram_tensor(nm, [n_], F32, kind="ExternalInput")
    io["yT"] = nc.dram_tensor("yT", [C, R], F32, kind="ExternalOutput")

    def bias_cols(name, n_):
        # bias vector [n_] -> sbuf [128, n_//128], col m = b[m*128:(m+1)*128]
        return bass.AP(tensor=io[name][:].tensor, offset=0, ap=[[1, P], [P, n_ // P]])

    with tile.TileContext(nc) as tc, ExitStack() as ctx:
        # ---- persistent sbuf pools
        const = ctx.enter_context(tc.tile_pool(name="const", bufs=1))
        x2p = ctx.enter_context(tc.tile_pool(name="x2p", bufs=KC))
        xn2p = ctx.enter_context(tc.tile_pool(name="xn2p", bufs=KC))
        otp = ctx.enter_context(tc.tile_pool(name="otp", bufs=K2))
        xrbp = ctx.enter_context(tc.tile_pool(name="xrbp", bufs=KC))
        wop = ctx.enter_context(tc.tile_pool(name="wop", bufs=K2))
        vecp = ctx.enter_context(tc.tile_pool(name="vecp", bufs=4))
        tmpp = ctx.enter_context(tc.tile_pool(name="tmpp", bufs=3))
        sqp = ctx.enter_context(tc.tile_pool(name="sqp", bufs=3))

        otd = [otp.tile([P, 2, R], FP8, tag="ot", name=f"ot{i}") for i in range(K2)]
        x2 = [x2p.tile([P, R], F32, tag="x2", name=f"x2_{i}") for i in range(KC)]
        xn2 = [xn2p.tile([P, R], BF16, tag="xn2", name=f"xn2_{i}") for i in range(KC)]
        xrb = [xrbp.tile([P, R], F32, tag="xrb", name=f"xrb{i}") for i in range(KC)]

        # ============== attention-lifetime scope ==============
        with tc.tile_pool(name="mskp", bufs=NT) as mskp, \
             tc.tile_pool(name="qtp", bufs=KC) as qtp, \
             tc.tile_pool(name="ktp", bufs=KC) as ktp, \
             tc.tile_pool(name="vtp", bufs=1) as vtp, \
             tc.tile_pool(name="xn1dp", bufs=K2) as xn1dp, \
             tc.tile_pool(name="wkvp", bufs=2 * K2) as wkvp, \
             tc.tile_pool(name="a2p", bufs=2) as a2p, \
             tc.tile_pool(name="recp", bufs=2) as recp, \
             tc.tile_pool(name="smr", bufs=2) as smr:

            qt = [qtp.tile([P, R], FP8, tag="qt", name=f"qt{i}") for i in range(KC)]
            kt = [ktp.tile([P, N], FP8, tag="kt", name=f"kt{i}") for i in range(KC)]
            vt_all = vtp.tile([P, K2, 2, H, D + 1], FP8, tag="vt", name="vt")
            xn1d = [xn1dp.tile([P, 2, N], FP8, tag="xn1", name=f"xn1_{i}")
                    for i in range(K2)]

            # ============== LN1 + Q scope ==============
            with tc.tile_pool(name="xs1", bufs=16) as xs1p, \
                 tc.tile_pool(name="wqp", bufs=K2) as wqp, \
                 tc.tile_pool(name="ln_ps", bufs=2, space="PSUM") as ln_ps, \
                 tc.tile_pool(name="ln_bc", bufs=2, space="PSUM") as ln_bc, \
                 tc.tile_pool(name="mm_ps", bufs=4, space="PSUM") as mm_ps:

                # ---- DMA issue order = priority: x, wq, msk, wk, wv, wo, xrb, biases
                xs1 = {}
                for chunk in range(2):
                    for k in range(KC):
                        xc = xs1p.tile([P, 512], F32R, tag="xs", name=f"xa_{chunk}_{k}")
                        eng = nc.gpsimd if k % 2 == 0 else nc.sync
                        eng.dma_start(out=xc[:],
                                      in_=io["xT"][k * P:(k + 1) * P,
                                                   chunk * 512:(chunk + 1) * 512])
                        xs1[(chunk, k)] = xc

                def load_w8(pool, name, tag):
                    tiles = []
                    for k2 in range(K2):
                        wt = pool.tile([P, 2, C], FP8, tag=tag, name=f"{name}_{k2}")
                        eng = nc.gpsimd if k2 % 2 == 0 else nc.sync
                        eng.dma_start(out=wt[:], in_=io[name][k2 * P:(k2 + 1) * P, :])
                        tiles.append(wt)
                    return tiles

                wq2 = load_w8(wqp, "wq8", "wq")

                msk_sb = [mskp.tile([P, R], FP8, tag="msk", name=f"msk{i}")
                          for i in range(NT)]
                for t in range(NT):
                    eng = nc.gpsimd if t % 2 == 0 else nc.sync
                    eng.dma_start(out=msk_sb[t][:], in_=io["mskT"][t * P:(t + 1) * P, :])

                wk2 = load_w8(wkvp, "wk8", "wkv")
                wv2 = load_w8(wkvp, "wv8", "wkv")
                wo2 = load_w8(wop, "wo8", "wo")

                for k in range(KC):
                    eng = nc.gpsimd if k % 2 == 0 else nc.sync
                    eng.dma_start(out=xrb[k][:], in_=io["xTb"][k * P:(k + 1) * P, :])

                # ---- constants
                bq_sb = const.tile([P, C // P], F32)
                bk_sb = const.tile([P, C // P], F32)
                b1_sb = const.tile([P, DFF // P], F32)
                b2_sb = const.tile([P, C // P], F32)
                nc.sync.dma_start(out=bq_sb[:], in_=bias_cols("bq", C))
                nc.sync.dma_start(out=bk_sb[:], in_=bias_cols("bk", C))
                nc.sync.dma_start(out=b1_sb[:], in_=bias_cols("b1", DFF))
                nc.sync.dma_start(out=b2_sb[:], in_=bias_cols("b2", C))
                ones_kf = const.tile([P, 1], F32)
                nc.vector.memset(ones_kf[:], 1.0)
                ones_kr = const.tile([P, 1], F32R)
                nc.vector.tensor_copy(out=ones_kr[:], in_=ones_kf[:])
                ones_cf = const.tile([1, P], F32)
                nc.vector.memset(ones_cf[:], 1.0)
                ones_c1 = const.tile([1, P], F32R)
                nc.vector.tensor_copy(out=ones_c1[:], in_=ones_cf[:])
                eps_sb = const.tile([1, 1], F32)
                nc.vector.memset(eps_sb[:], EPS)
                # sela/selb: broadcast sums of head-pair halves (for 1/sum)
                self_f = const.tile([1, P], F32)
                nc.vector.memset(self_f[:], 0.0)
                nc.vector.memset(self_f[0:1, 0:D], 1.0)
                sela = const.tile([1, P], F32R)
                nc.vector.tensor_copy(out=sela[:], in_=self_f[:])
                self_g = const.tile([1, P], F32)
                nc.vector.memset(self_g[:], 0.0)
                nc.vector.memset(self_g[0:1, D:2 * D], 1.0)
                selb = const.tile([1, P], F32R)
                nc.vector.tensor_copy(out=selb[:], in_=self_g[:])
                wupsrc = const.tile([P, 512], BF16)
                nc.vector.memset(wupsrc[:], 0.5)
                nc.vector.memset(vt_all[:, :, :, :, D:D + 1], 1.0)

                # HAM warmup while x streams in
                wup = ln_bc.tile([P, 512], F32, tag="lnbc", name="wup")
                for i in range(22):
                    nc.tensor.matmul(wup[:], wupsrc[:, 0:P], wupsrc[:],
                                     start=(i == 0), stop=(i == 21))
                wup_sb = vecp.tile([1, 1], F32, tag="vec", name="wupsb")
                nc.scalar.copy(wup_sb[:], wup[0:1, 0:1])

                # ---- LN1 stats + normalize -> xn1d fp8 (scale S_XN in rstd)
                inv_c = 1.0 / C
                for chunk in range(2):
                    ps_s = ln_ps.tile([1, 512], F32, tag="lnstat", name=f"l1s{chunk}")
                    ps_q = ln_ps.tile([1, 512], F32, tag="lnstat", name=f"l1q{chunk}")
                    for k in range(KC):
                        xc = xs1[(chunk, k)]
                        xf = xc[:].bitcast(F32)
                        sqc = sqp.tile([P, 512], F32R, tag="sq",
                                       name=f"sq{chunk}_{k}", bufs=2)
                        nc.vector.tensor_mul(sqc[:], xf, xf)
                        nc.tensor.matmul(ps_s[:], ones_kr[:], xc[:],
                                         start=(k == 0), stop=(k == KC - 1))
                        nc.tensor.matmul(ps_q[:], ones_kr[:], sqc[:],
                                         start=(k == 0), stop=(k == KC - 1))
                    mu = vecp.tile([1, 512], F32, tag="vec", name=f"l1mu{chunk}")
                    var = vecp.tile([1, 512], F32, tag="vec", name=f"l1var{chunk}")
                    rstd = vecp.tile([1, 512], F32, tag="vec", name=f"l1rstd{chunk}")
                    nc.scalar.mul(mu[:], ps_s[:], inv_c)
                    nc.scalar.mul(var[:], ps_q[:], inv_c)
                    msq = vecp.tile([1, 512], F32, tag="vec", name=f"l1msq{chunk}")
                    nc.vector.tensor_mul(msq[:], mu[:], mu[:])
                    nc.vector.tensor_sub(var[:], var[:], msq[:])
                    nc.scalar.activation(var[:], var[:], AF.Sqrt, bias=eps_sb[:])
                    nc.vector.reciprocal_approx_fast(out=rstd[:], in_=var[:])
                    mu_r = vecp.tile([1, 512], F32R, tag="vecr",
                                     name=f"l1mur{chunk}", bufs=2)
                    rstd_r = vecp.tile([1, 512], F32R, tag="vecr",
                                       name=f"l1rsr{chunk}", bufs=2)
                    nc.scalar.copy(mu_r[:], mu[:])
                    nc.scalar.mul(rstd_r[:], rstd[:], S_XN)
                    ps_mu = ln_bc.tile([P, 512], F32, tag="lnbc", name=f"l1bmu{chunk}")
                    ps_rstd = ln_bc.tile([P, 512], F32, tag="lnbc", name=f"l1brs{chunk}")
                    nc.tensor.matmul(ps_mu[:], ones_c1[:, 0:P], mu_r[:],
                                     start=True, stop=True)
                    nc.tensor.matmul(ps_rstd[:], ones_c1[:, 0:P], rstd_r[:],
                                     start=True, stop=True)
                    for k in range(KC):
                        xf = xs1[(chunk, k)][:].bitcast(F32)
                        t1 = tmpp.tile([P, 512], F32, tag="tmp", name=f"l1t{chunk}_{k}")
                        nc.vector.tensor_sub(t1[:], xf, ps_mu[:])
                        nc.vector.tensor_tensor(
                            xn1d[k // 2][:, k % 2, chunk * 512:(chunk + 1) * 512],
                            t1[:], ps_rstd[:], op=ALU.mult)

                # ---- Q projection (own tokens only): DoubleRow fp8, k2-outer
                for mg in range(2):
                    psq = [mm_ps.tile([P, 512], F32, tag="mm", name=f"q{mg}_{m}")
                           for m in range(4)]
                    for k2 in range(K2):
                        for m in range(4):
                            nc.tensor.matmul(
                                psq[m][:],
                                wq2[k2][:, :, (mg * 4 + m) * P:(mg * 4 + m + 1) * P],
                                xn1d[k2][:, :, 0:R],
                                start=(k2 == 0), stop=(k2 == K2 - 1), perf_mode=DR)
                    for m in range(4):
                        om = mg * 4 + m
                        nc.scalar.activation(qt[om][:], psq[m][:], AF.Identity,
                                             bias=bq_sb[:, om:om + 1],
                                             scale=1.0 / 128.0)

            # ============== K + V + attention pipeline ==============
            with tc.tile_pool(name="simps", bufs=2, space="PSUM") as simps, \
                 tc.tile_pool(name="ops", bufs=1, space="PSUM") as ops_, \
                 tc.tile_pool(name="mmb", bufs=2, space="PSUM") as mmb, \
                 tc.tile_pool(name="rbps", bufs=1, space="PSUM") as rbps:

                a_tiles = {}
                sums = {}

                def emit_k(p):
                    for nn_ in range(2):
                        ps = mmb.tile([P, 512], F32, tag="mm", name=f"kp{p}_{nn_}")
                        for k2 in range(K2):
                            nc.tensor.matmul(
                                ps[:], wk2[k2][:, :, p * P:(p + 1) * P],
                                xn1d[k2][:, :, nn_ * 512:(nn_ + 1) * 512],
                                start=(k2 == 0), stop=(k2 == K2 - 1), perf_mode=DR)
                        nc.scalar.activation(kt[p][:, nn_ * 512:(nn_ + 1) * 512],
                                             ps[:], AF.Identity,
                                             bias=bk_sb[:, p:p + 1], scale=1.0 / 512.0)

                def emit_v(u):
                    t, half = u % NT, u // NT
                    ps = mmb.tile([P, 512], F32, tag="mm", name=f"v{u}")
                    for k2 in range(K2):
                        nc.tensor.matmul(ps[:], xn1d[k2][:, :, t * P:(t + 1) * P],
                                         wv2[k2][:, :, half * 512:(half + 1) * 512],
                                         start=(k2 == 0), stop=(k2 == K2 - 1),
                                         perf_mode=DR)
                    dst = vt_all[:, t // 2, t % 2, half * 8:(half + 1) * 8, 0:D]
                    nc.scalar.activation(dst, ps[:].rearrange("p (h j) -> p h j", j=D),
                                         AF.Identity, scale=1.0 / 256.0)

                def emit_qk(p):
                    a2 = a2p.tile([P, K2, 2, 2, R], FP8, tag="a", name=f"a{p}")
                    a_tiles[p] = a2
                    for tk in range(NT):
                        ps2 = simps.tile([P, 2 * R], F32, tag="sim", name=f"s{p}_{tk}")
                        nc.tensor.matmul(ps2[:, 0:R], kt[p][0:D, tk * P:(tk + 1) * P],
                                         qt[p][0:D, :], start=True, stop=True)
                        nc.tensor.matmul(ps2[:, R:2 * R],
                                         kt[p][D:2 * D, tk * P:(tk + 1) * P],
                                         qt[p][D:2 * D, :], start=True, stop=True)
                        dst = a2[:, tk // 2, tk % 2, :, :]
                        nc.scalar.activation(dst,
                                             ps2[:].rearrange("p (h j) -> p h j", j=R),
                                             AF.Exp, scale=1.0 / 2048.0)
                        mrep = bass.AP(tensor=msk_sb[tk][:].tensor,
                                       offset=msk_sb[tk][:].offset,
                                       ap=[msk_sb[tk][:].ap[0], [0, 2], [1, R]])
                        nc.vector.tensor_tensor(dst, dst, mrep, op=ALU.mult)

                def emit_o(p):
                    a2 = a_tiles[p]
                    s0 = smr.tile([1, R], F32R, tag="s0", name=f"s0_{p}")
                    s1 = smr.tile([1, R], F32R, tag="s1", name=f"s1_{p}")
                    ot_tmp = recp.tile([P, R], BF16, tag="ott", name=f"ott{p}")
                    sums[p] = (s0, s1)
                    for hh in range(2):
                        h = 2 * p + hh
                        ps_o = ops_.tile([D + 1, R], F32, tag="o", name=f"o{h}")
                        for g in range(K2):
                            nc.tensor.matmul(ps_o[:], vt_all[:, g, :, h, :],
                                             a2[:, g, :, hh, :],
                                             start=(g == 0), stop=(g == K2 - 1),
                                             perf_mode=DR)
                        dst = s0 if hh == 0 else s1
                        nc.scalar.copy(dst[:], ps_o[D:D + 1, :])
                        nc.scalar.copy(ot_tmp[hh * D:(hh + 1) * D, :], ps_o[0:D, :])
                    a_tiles[p] = (a2, ot_tmp)

                def emit_norm(p):
                    s0, s1 = sums.pop(p)
                    a2, ot_tmp = a_tiles.pop(p)
                    ps_rb = rbps.tile([P, R], F32, tag="rb", name=f"rb{p}")
                    nc.tensor.matmul(ps_rb[:], sela[:], s0[:], start=True, stop=False)
                    nc.tensor.matmul(ps_rb[:], selb[:], s1[:], start=False, stop=True)
                    rec_sb = recp.tile([P, R], F32, tag="rec", name=f"rec{p}")
                    nc.vector.reciprocal_approx_fast(out=rec_sb[:], in_=ps_rb[:])
                    nc.vector.scalar_tensor_tensor(otd[p // 2][:, p % 2, :], ot_tmp[:],
                                                   float(S_O / S_V), rec_sb[:],
                                                   op0=ALU.mult, op1=ALU.mult)

                for p in range(H // 2):
                    emit_k(p)
                    if p < 4:
                        for j in range(4):
                            emit_v(4 * p + j)
                    if p > 0:
                        emit_o(p - 1)
                        emit_norm(p - 1)
                    emit_qk(p)
                emit_o(H // 2 - 1)
                emit_norm(H // 2 - 1)

        # ============== attn out proj + residual + LN2 + fc1 ==============
        with tc.tile_pool(name="w1p", bufs=16) as w1p, \
             tc.tile_pool(name="h1p", bufs=DFF // P) as h1p_outer, \
             tc.tile_pool(name="mm_ps2", bufs=4, space="PSUM") as mm_ps, \
             tc.tile_pool(name="ln_ps2", bufs=2, space="PSUM") as ln_ps, \
             tc.tile_pool(name="ln_bc2", bufs=2, space="PSUM") as ln_bc:

            h1p = h1p_outer
            h1 = [h1p.tile([P, R], BF16, tag="h1", name=f"h1_{i}")
                  for i in range(DFF // P)]

            ps_s = ln_ps.tile([1, 512], F32, tag="lnstat", name="l2s")
            ps_q = ln_ps.tile([1, 512], F32, tag="lnstat", name="l2q")
            for mg in range(2):
                pso = [mm_ps.tile([P, 512], F32, tag="mm", name=f"op{mg}_{m}")
                       for m in range(4)]
                for k2 in range(K2):
                    for m in range(4):
                        nc.tensor.matmul(
                            pso[m][:],
                            wo2[k2][:, :, (mg * 4 + m) * P:(mg * 4 + m + 1) * P],
                            otd[k2][:, :, :],
                            start=(k2 == 0), stop=(k2 == K2 - 1), perf_mode=DR)
                for m in range(4):
                    om = mg * 4 + m
                    nc.vector.scalar_tensor_tensor(x2[om][:], pso[m][:],
                                                   1.0 / (S_O * S_W), xrb[om][:],
                                                   op0=ALU.mult, op1=ALU.add)
                    sqc = sqp.tile([P, 512], F32R, tag="sq", name=f"sq2_{om}", bufs=2)
                    nc.vector.tensor_mul(sqc[:], x2[om][:], x2[om][:])
                    xcr = sqp.tile([P, 512], F32R, tag="xcr", name=f"x2r{om}", bufs=2)
                    nc.vector.tensor_copy(out=xcr[:], in_=x2[om][:])
                    nc.tensor.matmul(ps_s[:], ones_kr[:], xcr[:],
                                     start=(om == 0), stop=(om == KC - 1))
                    nc.tensor.matmul(ps_q[:], ones_kr[:], sqc[:],
                                     start=(om == 0), stop=(om == KC - 1))

            inv_c = 1.0 / C
            mu = vecp.tile([1, 512], F32, tag="vec", name="l2mu")
            var = vecp.tile([1, 512], F32, tag="vec", name="l2var")
            rstd = vecp.tile([1, 512], F32, tag="vec", name="l2rstd")
            nc.scalar.mul(mu[:], ps_s[:], inv_c)
            nc.scalar.mul(var[:], ps_q[:], inv_c)
            msq = vecp.tile([1, 512], F32, tag="vec", name="l2msq")
            nc.vector.tensor_mul(msq[:], mu[:], mu[:])
            nc.vector.tensor_sub(var[:], var[:], msq[:])
            nc.scalar.activation(var[:], var[:], AF.Sqrt, bias=eps_sb[:])
            nc.vector.reciprocal_approx_fast(out=rstd[:], in_=var[:])
            mu_r = vecp.tile([1, 512], F32R, tag="vecr", name="l2mur", bufs=2)
            rstd_r = vecp.tile([1, 512], F32R, tag="vecr", name="l2rsr", bufs=2)
            nc.scalar.copy(mu_r[:], mu[:])
            nc.scalar.copy(rstd_r[:], rstd[:])
            ps_mu = ln_bc.tile([P, 512], F32, tag="lnbc", name="l2bmu")
            ps_rstd = ln_bc.tile([P, 512], F32, tag="lnbc", name="l2brs")
            nc.tensor.matmul(ps_mu[:], ones_c1[:, 0:P], mu_r[:], start=True, stop=True)
            nc.tensor.matmul(ps_rstd[:], ones_c1[:, 0:P], rstd_r[:],
                             start=True, stop=True)
            for k in range(KC):
                t1 = tmpp.tile([P, 512], F32, tag="tmp", name=f"l2t{k}")
                nc.vector.tensor_sub(t1[:], x2[k][:], ps_mu[:])
                nc.vector.tensor_tensor(xn2[k][:], t1[:], ps_rstd[:], op=ALU.mult)

            # ---- fc1 (bf16), k-outer to chase LN2 normalize
            for cg in range(4):
                w1_sb = [w1p.tile([P, C], BF16, tag="w1", name=f"w1_{cg}_{i}")
                         for i in range(KC)]
                for k in range(KC):
                    eng = nc.gpsimd if k % 2 == 0 else nc.sync
                    eng.dma_start(out=w1_sb[k][:],
                                  in_=io["w1"][k * P:(k + 1) * P, cg * C:(cg + 1) * C])
                for mg in range(2):
                    psf = [mm_ps.tile([P, 512], F32, tag="mm", name=f"f{cg}{mg}_{m}")
                           for m in range(4)]
                    for k in range(KC):
                        for m in range(4):
                            nc.tensor.matmul(
                                psf[m][:],
                                w1_sb[k][:, (mg * 4 + m) * P:(mg * 4 + m + 1) * P],
                                xn2[k][:],
                                start=(k == 0), stop=(k == KC - 1))
                    for m in range(4):
                        om = cg * 8 + mg * 4 + m
                        nc.scalar.activation(h1[om][:], psf[m][:], AF.Gelu_apprx_tanh,
                                             bias=b1_sb[:, om:om + 1])

            # ---- fc2 (bf16) + residual, output-half groups like baseline
            with tc.tile_pool(name="w2p", bufs=DFF // P) as w2p, \
                 tc.tile_pool(name="yp", bufs=2) as yp:
                for mg in range(2):
                    w2_sb = [w2p.tile([P, 512], BF16, tag="w2", name=f"w2_{mg}_{i}")
                             for i in range(DFF // P)]
                    for k in range(DFF // P):
                        eng = nc.gpsimd if k % 2 == 0 else nc.sync
                        eng.dma_start(out=w2_sb[k][:],
                                      in_=io["w2"][k * P:(k + 1) * P,
                                                   mg * 512:(mg + 1) * 512])
                    psy = [mm_ps.tile([P, 512], F32, tag="mm", name=f"y{mg}_{m}")
                           for m in range(4)]
                    for k in range(DFF // P):
                        for m in range(4):
                            nc.tensor.matmul(psy[m][:],
                                             w2_sb[k][:, m * P:(m + 1) * P],
                                             h1[k][:],
                                             start=(k == 0), stop=(k == DFF // P - 1))
                    for m in range(4):
                        om = mg * 4 + m
                        y_sb = yp.tile([P, R], F32, tag="y", name=f"y{om}")
                        nc.vector.scalar_tensor_tensor(y_sb[:], psy[m][:],
                                                       b2_sb[:, om:om + 1], x2[om][:],
                                                       op0=ALU.add, op1=ALU.add)
                        eng = nc.gpsimd if om % 2 == 0 else nc.sync
                        eng.dma_start(out=io["yT"][om * P:(om + 1) * P, :], in_=y_sb[:])

    if not nc.is_finalized():
        nc.finalize()
    return nc


def _get_nc():
    if "nc" not in _CACHE:
        _CACHE["nc"] = _build()
    return _CACHE["nc"]


def _q8(w, scale):
    return np.clip(np.asarray(w, np.float32) * scale, -224.0, 224.0).astype(F8NP)


def _dr_pack(w8):
    # [C_in, M] -> [C_in//2, 2*M]: row r=k2*128+p holds the (i=0, i=1) k-tile pair
    K, M = w8.shape
    return np.ascontiguousarray(
        w8.reshape(K // 256, 2, P, M).transpose(0, 2, 1, 3).reshape(K // 2, 2 * M))


def _prep_in_maps(inputs):
    x = np.asarray(inputs["x"], dtype=np.float32)
    mask = np.asarray(inputs["mask"])
    scale = float(D) ** -0.5
    wq = np.asarray(inputs["wq"], np.float32) * scale
    bq = np.asarray(inputs["bq"], np.float32) * scale
    wkv = np.asarray(inputs["wkv"], np.float32)
    bkv = np.asarray(inputs["bkv"], np.float32)
    wk = np.ascontiguousarray(wkv[:, :C])
    wv = np.ascontiguousarray(wkv[:, C:])
    bk = np.ascontiguousarray(bkv[:C])
    bv = np.ascontiguousarray(bkv[C:])
    wo = np.asarray(inputs["wo"], np.float32)
    bo = np.asarray(inputs["bo"], np.float32)
    w1 = np.asarray(inputs["w1"], np.float32).astype(BFNP)
    b1 = np.asarray(inputs["b1"], np.float32)
    w2 = np.asarray(inputs["w2"], np.float32).astype(BFNP)
    b2 = np.asarray(inputs["b2"], np.float32)

    wq8 = _dr_pack(_q8(wq, S_WQ))
    wk8 = _dr_pack(_q8(wk, S_W))
    wv8 = _dr_pack(_q8(wv, S_W))
    wo8 = _dr_pack(_q8(wo, S_W))
    bo_f = (bo.astype(np.float64) + bv.astype(np.float64) @ wo.astype(np.float64))
    mask8 = mask.astype(np.float32).astype(F8NP)

    shared = dict(wq8=wq8, wk8=wk8, wv8=wv8, wo8=wo8, w1=w1, w2=w2,
                  bq=(bq * S_Q).astype(np.float32), bk=(bk * S_K).astype(np.float32),
                  b1=b1, b2=b2)
    in_maps = []
    for c in range(8):
        b = c // 2
        rh = c % 2
        own = np.arange(rh * R, rh * R + R)
        oth = np.arange((1 - rh) * R, (1 - rh) * R + R)
        perm = np.concatenate([own, oth])
        xT = np.ascontiguousarray(x[b].T[:, perm])
        xTb = np.ascontiguousarray(xT[:, 0:R] + bo_f[:, None].astype(np.float32))
        mskT = np.ascontiguousarray(mask8[np.ix_(own, perm)].T)
        m = dict(shared)
        m["xT"] = xT
        m["xTb"] = xTb.astype(np.float32)
        m["mskT"] = mskT
        in_maps.append(m)
    return in_maps


def _assemble(results):
    out = np.empty((B, N, C), dtype=np.float32)
    for c in range(8):
        b = c // 2
        rh = c % 2
        out[b, rh * R:(rh + 1) * R, :] = results[c]["yT"].T
    return out


def run(inputs, trace=False):
    nc = _get_nc()
    in_maps = _prep_in_maps(inputs)
    res = run_bass_kernel_spmd(nc, in_maps, core_ids=list(range(8)), trace=trace)
    return _assemble(res.results), res


def kernel(**inputs):
    out, _ = run(inputs, trace=False)
    return out
